# revision 5
# baseline (speedup 1.0000x reference)
import math
import os
import sys

sys.path.insert(0, "/opt/trn_rl_repo")

import numpy as np

import concourse.bass as bass
import concourse.tile as tile
from concourse import bacc, mybir
from concourse.bass_utils import run_bass_kernel_spmd

# Problem constants (nn_AttentionBlock: B=4, C=512, H=W=48, 8 heads, GN32)
B = 4
C = 512
T = 2304  # 48*48
NH = 8
CH = 64  # C // NH
NG = 32  # groupnorm groups
GS = 16  # channels per group
EPS = 1e-5
S_ATT = 1.0 / math.sqrt(math.sqrt(CH))

N_CORES = 8
HPC = 4  # heads per core
CT = 4  # channel tiles of 128
KT = 4  # contraction tiles of 128 over C
ST = 18  # sequence tiles of 128 over T
SG = 3  # s-tiles per exp group
NGRP = ST // SG
CHUNKS = [512, 512, 512, 512, 256]  # t chunks (sum = 2304)

F32 = mybir.dt.float32
F32R = mybir.dt.float32r
BF16 = mybir.dt.bfloat16

_CACHE = {}


def _build():
    nc = bacc.Bacc("TRN2", target_bir_lowering=False, debug=False,
                   enable_asserts=True, num_devices=N_CORES)

    xb = nc.dram_tensor("xb", [C, T], F32, kind="ExternalInput").ap()
    qkwT = nc.dram_tensor("qkwT", [C, 512], F32R, kind="ExternalInput").ap()
    vwT = nc.dram_tensor("vwT", [C, 256], F32R, kind="ExternalInput").ap()
    projwT = nc.dram_tensor("projwT", [256, C], F32R, kind="ExternalInput").ap()
    qkb = nc.dram_tensor("qkb", [4, 128], F32, kind="ExternalInput").ap()
    gnw = nc.dram_tensor("gnw", [CT, 128], F32, kind="ExternalInput").ap()
    gnb = nc.dram_tensor("gnb", [CT, 128], F32, kind="ExternalInput").ap()
    ind = nc.dram_tensor("ind", [128, 8], F32, kind="ExternalInput").ap()
    indT = nc.dram_tensor("indT", [8, 128], F32, kind="ExternalInput").ap()
    resscale = nc.dram_tensor("resscale", [CT, 128], F32, kind="ExternalInput").ap()
    pbeff = nc.dram_tensor("pbeff", [CT, 128], F32, kind="ExternalInput").ap()
    out = nc.dram_tensor("out", [C, T], F32, kind="ExternalOutput").ap()
    dbg = os.environ.get("KBG_DEBUG")
    if dbg:
        d_xn = nc.dram_tensor("d_xn", [CT * 128, T], F32, kind="ExternalOutput").ap()
        d_qk = nc.dram_tensor("d_qk", [128, 2, 2, T], F32, kind="ExternalOutput").ap()
        d_vt = nc.dram_tensor("d_vt", [128, ST, HPC, 128], F32, kind="ExternalOutput").ap()
        d_gs = nc.dram_tensor("d_gs", [128, CT, 3], F32, kind="ExternalOutput").ap()

    with tile.TileContext(nc) as tc:
        with (
            tc.tile_pool(name="persist", bufs=1) as persist,
            tc.tile_pool(name="xin", bufs=2) as xin,
            tc.tile_pool(name="wexp", bufs=4) as wexpp,
            tc.tile_pool(name="small", bufs=4) as small,
            tc.tile_pool(name="ap", bufs=2) as ap_pool,
            tc.tile_pool(name="outp", bufs=2) as outp,
            tc.tile_pool(name="rcp", bufs=2) as rcp,
            tc.tile_pool(name="pqk", bufs=2, space="PSUM") as pqk,
            tc.tile_pool(name="pmm", bufs=2, space="PSUM") as pmm,
        ):
            # ---- persistent SBUF state ----
            xn_all = persist.tile([128, CT, T], F32R)        # normalized x
            qk_all = persist.tile([128, 2, 2, T], BF16)      # [qA|qB / kA|kB] per pair
            vT_all = persist.tile([128, ST, HPC, 128], F32R)  # [vT(64) | ones(64)] per head
            w_qk = persist.tile([128, KT, 512], F32R)
            w_v = persist.tile([128, KT, 256], F32R)
            w_pj = persist.tile([128, 2, 512], F32R)
            sb_qkb = persist.tile([128, 4], F32)
            sb_gnw = persist.tile([128, CT], F32)
            sb_gnb = persist.tile([128, CT], F32)
            sb_rs = persist.tile([128, CT], F32)
            sb_pb = persist.tile([128, CT], F32)
            sb_ind = persist.tile([128, 8], F32)
            sb_indT = persist.tile([8, 128], F32)

            # weight / constant loads
            for kt in range(KT):
                nc.sync.dma_start(w_qk[:, kt, :], qkwT[128 * kt:128 * (kt + 1), :])
                nc.sync.dma_start(w_v[:, kt, :], vwT[128 * kt:128 * (kt + 1), :])
            for j in range(2):
                nc.sync.dma_start(w_pj[:, j, :], projwT[128 * j:128 * (j + 1), :])
            # [4,128] dram -> [128,4] sbuf (transpose via AP)
            for (dst, src) in ((sb_qkb, qkb), (sb_gnw, gnw), (sb_gnb, gnb),
                               (sb_rs, resscale), (sb_pb, pbeff)):
                nc.sync.dma_start(dst[:, :], src.transpose([1, 0]))
            nc.sync.dma_start(sb_ind[:, :], ind[:, :])
            nc.sync.dma_start(sb_indT[:, :], indT[:, :])
            # ones columns of vT_all
            nc.vector.memset(vT_all[:, :, :, 64:128].bitcast(F32), 1.0)

            # ---- phase 1: groupnorm -> xn_all ----
            for ct in range(CT):
                x_t = xin.tile([128, T], F32, tag="x")
                nc.sync.dma_start(x_t[:, :], xb[128 * ct:128 * (ct + 1), :])

                stats = small.tile([128, 9, 6], F32, tag="stats")
                for j in range(9):
                    nc.vector.bn_stats(out=stats[:, j, :], in_=x_t[:, 256 * j:256 * (j + 1)])
                mv = small.tile([128, 2], F32, tag="mv")
                nc.vector.bn_aggr(out=mv[:, :], in_=stats[:, :, :])

                em2 = small.tile([128, 2], F32, tag="em2")
                nc.vector.tensor_copy(em2[:, 0:1], mv[:, 0:1])
                nc.vector.tensor_tensor(out=em2[:, 1:2], in0=mv[:, 0:1], in1=mv[:, 0:1],
                                        op=mybir.AluOpType.mult)
                nc.vector.tensor_add(em2[:, 1:2], em2[:, 1:2], mv[:, 1:2])

                gsum = pmm.tile([8, 2], F32, tag="mm")
                nc.tensor.matmul(gsum[:, :], sb_ind[:, :], em2[:, :], start=True, stop=True)

                # group stats -> [mu, rstd] with a Newton-refined rsqrt
                gst = small.tile([8, 2], F32, tag="gst")
                v_t = small.tile([8, 5], F32, tag="gtmp")
                nc.vector.tensor_scalar_mul(gst[:, 0:1], gsum[:, 0:1], 1.0 / GS)
                nc.vector.tensor_scalar(out=v_t[:, 0:1], in0=gsum[:, 1:2],
                                        scalar1=1.0 / GS, scalar2=EPS,
                                        op0=mybir.AluOpType.mult,
                                        op1=mybir.AluOpType.add)
                # v holds E[x^2]+eps; subtract mu^2 -> var+eps
                nc.vector.tensor_tensor(out=v_t[:, 1:2], in0=gst[:, 0:1], in1=gst[:, 0:1],
                                        op=mybir.AluOpType.mult)
                nc.vector.tensor_sub(v_t[:, 0:1], v_t[:, 0:1], v_t[:, 1:2])
                nc.scalar.activation(out=v_t[:, 2:3], in_=v_t[:, 0:1],
                                     func=mybir.ActivationFunctionType.Sqrt)
                nc.vector.reciprocal(v_t[:, 3:4], v_t[:, 2:3])  # r0 ~ rsqrt
                nc.vector.tensor_tensor(out=v_t[:, 4:5], in0=v_t[:, 3:4], in1=v_t[:, 3:4],
                                        op=mybir.AluOpType.mult)  # r0^2
                nc.vector.tensor_tensor(out=v_t[:, 4:5], in0=v_t[:, 4:5], in1=v_t[:, 0:1],
                                        op=mybir.AluOpType.mult)  # v*r0^2
                nc.vector.tensor_scalar(out=v_t[:, 4:5], in0=v_t[:, 4:5],
                                        scalar1=-0.5, scalar2=1.5,
                                        op0=mybir.AluOpType.mult,
                                        op1=mybir.AluOpType.add)  # 1.5 - 0.5*v*r0^2
                nc.vector.tensor_tensor(out=gst[:, 1:2], in0=v_t[:, 3:4], in1=v_t[:, 4:5],
                                        op=mybir.AluOpType.mult)  # refined rstd

                bc = pmm.tile([128, 2], F32, tag="mm")
                nc.tensor.matmul(bc[:, :], sb_indT[:, :], gst[:, :], start=True, stop=True)

                sv = small.tile([128, 3], F32, tag="sv")
                nc.vector.tensor_tensor(out=sv[:, 0:1], in0=bc[:, 1:2],
                                        in1=sb_gnw[:, ct:ct + 1], op=mybir.AluOpType.mult)
                nc.vector.tensor_tensor(out=sv[:, 1:2], in0=bc[:, 0:1], in1=sv[:, 0:1],
                                        op=mybir.AluOpType.mult)
                nc.vector.tensor_tensor(out=sv[:, 2:3], in0=sb_gnb[:, ct:ct + 1],
                                        in1=sv[:, 1:2], op=mybir.AluOpType.subtract)
                nc.vector.tensor_scalar(out=xn_all[:, ct, :], in0=x_t[:, :],
                                        scalar1=sv[:, 0:1], scalar2=sv[:, 2:3],
                                        op0=mybir.AluOpType.mult,
                                        op1=mybir.AluOpType.add)
                if dbg:
                    dgs = small.tile([128, 3], F32, tag="dgs")
                    nc.vector.tensor_copy(dgs[:, :], sv[:, :])
                    nc.sync.dma_start(d_gs[:, ct, :], dgs[:, :])
                    dxn = xin.tile([128, T], F32, tag="dxn")
                    nc.vector.tensor_copy(dxn[:, :], xn_all[:, ct, :])
                    nc.sync.dma_start(d_xn[128 * ct:128 * (ct + 1), :], dxn[:, :])

            # ---- phase 2: q, k ----
            for p_ in range(2):
                for part in range(2):  # 0=q, 1=k
                    t0 = 0
                    for n in CHUNKS:
                        ps = pmm.tile([128, 512], F32, tag="mm")
                        for kt in range(KT):
                            nc.tensor.matmul(
                                ps[:, 0:n],
                                w_qk[:, kt, 256 * p_ + 128 * part:256 * p_ + 128 * part + 128],
                                xn_all[:, kt, t0:t0 + n],
                                start=(kt == 0), stop=(kt == KT - 1))
                        nc.vector.tensor_scalar_add(
                            qk_all[:, p_, part, t0:t0 + n], ps[:, 0:n],
                            sb_qkb[:, 2 * p_ + part:2 * p_ + part + 1])
                        t0 += n

            if dbg:
                for p_ in range(2):
                    for part in range(2):
                        dq = xin.tile([128, T], F32, tag="dxn")
                        nc.vector.tensor_copy(dq[:, :], qk_all[:, p_, part, :])
                        nc.sync.dma_start(d_qk[:, p_, part, :], dq[:, :])

            # ---- phase 3: vT (v computed transposed) ----
            for st in range(ST):
                ps = pmm.tile([128, 256], F32, tag="mm")
                for kt in range(KT):
                    nc.tensor.matmul(ps[:, :], xn_all[:, kt, 128 * st:128 * (st + 1)],
                                     w_v[:, kt, :], start=(kt == 0), stop=(kt == KT - 1))
                nc.vector.tensor_copy(
                    vT_all[:, st, :, 0:64],
                    ps[:, :].rearrange("p (h c) -> p h c", h=HPC))

            if dbg:
                for st in range(ST):
                    dv = small.tile([128, HPC, 128], F32, tag="dvt")
                    nc.vector.tensor_copy(dv[:, :, :], vT_all[:, st, :, :])
                    nc.sync.dma_start(d_vt[:, st, :, :], dv[:, :, :])

            # ---- phase 4: attention + proj per t-chunk ----
            t0 = 0
            for ci, n in enumerate(CHUNKS):
                a0 = ap_pool.tile([128, 512], F32R, tag="a0")
                a1 = ap_pool.tile([128, 512], F32R, tag="a1")
                a_t = (a0, a0, a1, a1)
                for h in range(HPC):
                    p_, hh = h // 2, h % 2
                    q_sl = qk_all[64 * hh:64 * hh + 64, p_, 0, t0:t0 + n]
                    av = pmm.tile([128, 512], F32, tag="mm")
                    for g in range(NGRP):
                        qs = pqk.tile([128, SG, 512], F32, tag="qk")
                        for i3 in range(SG):
                            st = SG * g + i3
                            nc.tensor.matmul(
                                qs[:, i3, 0:n],
                                qk_all[64 * hh:64 * hh + 64, p_, 1,
                                       128 * st:128 * (st + 1)],
                                q_sl, start=True, stop=True)
                        we = wexpp.tile([128, SG, 512], F32R, tag="we")
                        nc.scalar.activation(out=we[:, :, 0:n], in_=qs[:, :, 0:n],
                                             func=mybir.ActivationFunctionType.Exp)
                        for i3 in range(SG):
                            st = SG * g + i3
                            nc.tensor.matmul(av[:, 0:n], vT_all[:, st, h, :],
                                             we[:, i3, 0:n],
                                             start=(st == 0), stop=(st == ST - 1))
                    rc = rcp.tile([64, 512], F32, tag="rc")
                    nc.vector.reciprocal(rc[:, 0:n], av[64:128, 0:n])
                    nc.vector.tensor_tensor(out=a_t[h][64 * hh:64 * hh + 64, 0:n],
                                            in0=av[0:64, 0:n], in1=rc[:, 0:n],
                                            op=mybir.AluOpType.mult)

                for ot in range(CT):
                    pp = pmm.tile([128, 512], F32, tag="mm")
                    nc.tensor.matmul(pp[:, 0:n], w_pj[:, 0, 128 * ot:128 * (ot + 1)],
                                     a0[:, 0:n], start=True, stop=False)
                    nc.tensor.matmul(pp[:, 0:n], w_pj[:, 1, 128 * ot:128 * (ot + 1)],
                                     a1[:, 0:n], start=False, stop=True)
                    t1 = outp.tile([128, 512], F32, tag="t1")
                    nc.vector.tensor_scalar(out=t1[:, 0:n], in0=xn_all[:, ot, t0:t0 + n],
                                            scalar1=sb_rs[:, ot:ot + 1],
                                            scalar2=sb_pb[:, ot:ot + 1],
                                            op0=mybir.AluOpType.mult,
                                            op1=mybir.AluOpType.add)
                    o_t = outp.tile([128, 512], F32, tag="out")
                    nc.vector.tensor_tensor(out=o_t[:, 0:n], in0=pp[:, 0:n],
                                            in1=t1[:, 0:n], op=mybir.AluOpType.add)
                    nc.sync.dma_start(out[128 * ot:128 * (ot + 1), t0:t0 + n],
                                      o_t[:, 0:n])
                t0 += n

    nc.compile()
    return nc


def _shard(inputs):
    x = np.ascontiguousarray(np.asarray(inputs["x"], dtype=np.float32))
    gn_w = np.asarray(inputs["gn_w"], dtype=np.float32)
    gn_b = np.asarray(inputs["gn_b"], dtype=np.float32)
    qkv_w = np.asarray(inputs["qkv_w"], dtype=np.float32)
    qkv_b = np.asarray(inputs["qkv_b"], dtype=np.float32)
    proj_w = np.asarray(inputs["proj_w"], dtype=np.float32)
    proj_b = np.asarray(inputs["proj_b"], dtype=np.float32)

    ind = np.zeros((128, 8), np.float32)
    for c in range(128):
        ind[c, c // GS] = 1.0
    indT = np.ascontiguousarray(ind.T)
    gnw4 = np.ascontiguousarray(gn_w.reshape(CT, 128))
    gnb4 = np.ascontiguousarray(gn_b.reshape(CT, 128))

    in_maps = []
    for core in range(N_CORES):
        b, hg = core // 2, core % 2
        xb = np.ascontiguousarray(x[b].reshape(C, T))

        qkwT = np.empty((C, 512), np.float32)
        qkbm = np.empty((4, 128), np.float32)
        for p_ in range(2):
            for part in range(2):
                for j in range(2):
                    gh = 4 * hg + 2 * p_ + j
                    rows = qkv_w[192 * gh + 64 * part:192 * gh + 64 * part + 64, :]
                    col0 = 256 * p_ + 128 * part + 64 * j
                    qkwT[:, col0:col0 + 64] = (rows * S_ATT).T
                    qkbm[2 * p_ + part, 64 * j:64 * j + 64] = \
                        qkv_b[192 * gh + 64 * part:192 * gh + 64 * part + 64] * S_ATT

        vwT = np.empty((C, 256), np.float32)
        bv = np.empty((256,), np.float32)
        for j in range(HPC):
            gh = 4 * hg + j
            vwT[:, 64 * j:64 * j + 64] = qkv_w[192 * gh + 128:192 * gh + 192, :].T
            bv[64 * j:64 * j + 64] = qkv_b[192 * gh + 128:192 * gh + 192]

        pw = proj_w[:, 256 * hg:256 * hg + 256]
        projwT = np.ascontiguousarray(pw.T)
        pb = pw @ bv
        if hg == 0:
            pb = pb + proj_b
        rs = np.full((CT, 128), 1.0 if hg == 0 else 0.0, np.float32)

        in_maps.append({
            "xb": xb,
            "qkwT": np.ascontiguousarray(qkwT),
            "vwT": np.ascontiguousarray(vwT),
            "projwT": projwT,
            "qkb": np.ascontiguousarray(qkbm),
            "gnw": gnw4, "gnb": gnb4,
            "ind": ind, "indT": indT,
            "resscale": rs,
            "pbeff": np.ascontiguousarray(pb.reshape(CT, 128).astype(np.float32)),
        })
    return in_maps


def _ensure_ntff_hook():
    """Install the antenv.axon_hooks shim so BASS_TRACE=1 can capture NTFF
    profiles through libaxon_pjrt.so (the image ships the .so but not the
    python-side hook module)."""
    try:
        from antenv.axon_hooks import get_axon_ntff_profile_hook  # noqa: F401
        return
    except ImportError:
        pass
    import types

    try:
        import antenv
        from trn_agent_boot.trn_boot import _ntff_profile_via_ctypes
    except ImportError:
        return
    mod = types.ModuleType("antenv.axon_hooks")
    holder = {"hook": None}
    mod.set_axon_ntff_profile_hook = lambda h: holder.__setitem__("hook", h)
    mod.get_axon_ntff_profile_hook = lambda: holder["hook"]
    sys.modules["antenv.axon_hooks"] = mod
    antenv.axon_hooks = mod
    so = "/opt/axon/libaxon_pjrt.so"
    if os.path.exists(so):
        hook = _ntff_profile_via_ctypes(so)
        if hook is not None:
            mod.set_axon_ntff_profile_hook(hook)


LAST_RESULTS = None


def kernel(**inputs):
    global LAST_RESULTS
    if os.environ.get("BASS_TRACE"):
        _ensure_ntff_hook()
    if "nc" not in _CACHE:
        _CACHE["nc"] = _build()
    nc = _CACHE["nc"]
    in_maps = _shard(inputs)
    res = run_bass_kernel_spmd(nc, in_maps, core_ids=list(range(N_CORES)))
    LAST_RESULTS = res
    out = np.empty((B, C, T), np.float32)
    for b in range(B):
        out[b] = res.results[2 * b]["out"] + res.results[2 * b + 1]["out"]
    return out.reshape(B, C, 48, 48)


# revision 7
# speedup vs baseline: 1.0992x; 1.0992x over previous
import math
import os
import sys

sys.path.insert(0, "/opt/trn_rl_repo")

import numpy as np

import concourse.bass as bass
import concourse.tile as tile
from concourse import bacc, mybir
from concourse.bass_utils import run_bass_kernel_spmd

# Problem constants (nn_AttentionBlock: B=4, C=512, H=W=48, 8 heads, GN32)
B = 4
C = 512
T = 2304  # 48*48
NH = 8
CH = 64  # C // NH
NG = 32  # groupnorm groups
GS = 16  # channels per group
EPS = 1e-5
S_ATT = 1.0 / math.sqrt(math.sqrt(CH))

N_CORES = 8
HPC = 4  # heads per core
CT = 4  # channel tiles of 128
KT = 4  # contraction tiles of 128 over C
ST = 18  # sequence tiles of 128 over T
SG = 3  # s-tiles per exp group
NGRP = ST // SG
CHUNKS = [512, 512, 512, 512, 256]  # t chunks (sum = 2304)

F32 = mybir.dt.float32
F32R = mybir.dt.float32r
BF16 = mybir.dt.bfloat16

_CACHE = {}


def _build():
    nc = bacc.Bacc("TRN2", target_bir_lowering=False, debug=False,
                   enable_asserts=True, num_devices=N_CORES)

    xb = nc.dram_tensor("xb", [C, T], F32, kind="ExternalInput").ap()
    qkwT = nc.dram_tensor("qkwT", [C, 512], F32R, kind="ExternalInput").ap()
    vwT = nc.dram_tensor("vwT", [C, 256], F32R, kind="ExternalInput").ap()
    projwT = nc.dram_tensor("projwT", [256, C], F32R, kind="ExternalInput").ap()
    qkb = nc.dram_tensor("qkb", [4, 128], F32, kind="ExternalInput").ap()
    gnw = nc.dram_tensor("gnw", [CT, 128], F32, kind="ExternalInput").ap()
    gnb = nc.dram_tensor("gnb", [CT, 128], F32, kind="ExternalInput").ap()
    ind = nc.dram_tensor("ind", [128, 8], F32, kind="ExternalInput").ap()
    indT = nc.dram_tensor("indT", [8, 128], F32, kind="ExternalInput").ap()
    resscale = nc.dram_tensor("resscale", [CT, 128], F32, kind="ExternalInput").ap()
    pbeff = nc.dram_tensor("pbeff", [CT, 128], F32, kind="ExternalInput").ap()
    out = nc.dram_tensor("out", [C, T], F32, kind="ExternalOutput").ap()
    dbg = os.environ.get("KBG_DEBUG")
    if dbg:
        d_xn = nc.dram_tensor("d_xn", [CT * 128, T], F32, kind="ExternalOutput").ap()
        d_qk = nc.dram_tensor("d_qk", [128, 2, 2, T], F32, kind="ExternalOutput").ap()
        d_vt = nc.dram_tensor("d_vt", [128, ST, HPC, 128], F32, kind="ExternalOutput").ap()
        d_gs = nc.dram_tensor("d_gs", [128, CT, 3], F32, kind="ExternalOutput").ap()

    with tile.TileContext(nc) as tc:
        with (
            tc.tile_pool(name="persist", bufs=1) as persist,
            tc.tile_pool(name="xin", bufs=2) as xin,
            tc.tile_pool(name="wexp", bufs=5) as wexpp,
            tc.tile_pool(name="small", bufs=4) as small,
            tc.tile_pool(name="ap", bufs=2) as ap_pool,
            tc.tile_pool(name="outp", bufs=2) as outp,
            tc.tile_pool(name="rcp", bufs=2) as rcp,
            tc.tile_pool(name="pqk", bufs=2, space="PSUM") as pqk,
            tc.tile_pool(name="pmm", bufs=2, space="PSUM") as pmm,
        ):
            # ---- persistent SBUF state ----
            xn_all = persist.tile([128, CT, T], F32R)        # normalized x
            qk_all = persist.tile([128, 2, 2, T], BF16)      # [qA|qB / kA|kB] per pair
            vT_all = persist.tile([128, ST, HPC, 128], F32R)  # [vT(64) | ones(64)] per head
            w_qk = persist.tile([128, KT, 512], F32R)
            w_v = persist.tile([128, KT, 256], F32R)
            w_pj = persist.tile([128, 2, 512], F32R)
            sb_qkb = persist.tile([128, 4], F32)
            sb_gnw = persist.tile([128, CT], F32)
            sb_gnb = persist.tile([128, CT], F32)
            sb_rs = persist.tile([128, CT], F32)
            sb_pb = persist.tile([128, CT], F32)
            sb_ind = persist.tile([128, 8], F32)
            sb_indT = persist.tile([8, 128], F32)

            # weight / constant loads
            for kt in range(KT):
                nc.sync.dma_start(w_qk[:, kt, :], qkwT[128 * kt:128 * (kt + 1), :])
                nc.sync.dma_start(w_v[:, kt, :], vwT[128 * kt:128 * (kt + 1), :])
            for j in range(2):
                nc.sync.dma_start(w_pj[:, j, :], projwT[128 * j:128 * (j + 1), :])
            # [4,128] dram -> [128,4] sbuf (transpose via AP)
            for (dst, src) in ((sb_qkb, qkb), (sb_gnw, gnw), (sb_gnb, gnb),
                               (sb_rs, resscale), (sb_pb, pbeff)):
                nc.sync.dma_start(dst[:, :], src.transpose([1, 0]))
            nc.sync.dma_start(sb_ind[:, :], ind[:, :])
            nc.sync.dma_start(sb_indT[:, :], indT[:, :])
            # ones columns of vT_all
            nc.vector.memset(vT_all[:, :, :, 0:64].bitcast(F32), 1.0)

            # ---- phase 1: groupnorm -> xn_all ----
            for ct in range(CT):
                x_t = xin.tile([128, T], F32, tag="x")
                nc.sync.dma_start(x_t[:, :], xb[128 * ct:128 * (ct + 1), :])

                stats = small.tile([128, 9, 6], F32, tag="stats")
                for j in range(9):
                    nc.vector.bn_stats(out=stats[:, j, :], in_=x_t[:, 256 * j:256 * (j + 1)])
                mv = small.tile([128, 2], F32, tag="mv")
                nc.vector.bn_aggr(out=mv[:, :], in_=stats[:, :, :])

                em2 = small.tile([128, 2], F32, tag="em2")
                nc.vector.tensor_copy(em2[:, 0:1], mv[:, 0:1])
                nc.vector.tensor_tensor(out=em2[:, 1:2], in0=mv[:, 0:1], in1=mv[:, 0:1],
                                        op=mybir.AluOpType.mult)
                nc.vector.tensor_add(em2[:, 1:2], em2[:, 1:2], mv[:, 1:2])

                gsum = pmm.tile([8, 2], F32, tag="mm")
                nc.tensor.matmul(gsum[:, :], sb_ind[:, :], em2[:, :], start=True, stop=True)

                # group stats -> [mu, rstd] with a Newton-refined rsqrt
                gst = small.tile([8, 2], F32, tag="gst")
                v_t = small.tile([8, 5], F32, tag="gtmp")
                nc.vector.tensor_scalar_mul(gst[:, 0:1], gsum[:, 0:1], 1.0 / GS)
                nc.vector.tensor_scalar(out=v_t[:, 0:1], in0=gsum[:, 1:2],
                                        scalar1=1.0 / GS, scalar2=EPS,
                                        op0=mybir.AluOpType.mult,
                                        op1=mybir.AluOpType.add)
                # v holds E[x^2]+eps; subtract mu^2 -> var+eps
                nc.vector.tensor_tensor(out=v_t[:, 1:2], in0=gst[:, 0:1], in1=gst[:, 0:1],
                                        op=mybir.AluOpType.mult)
                nc.vector.tensor_sub(v_t[:, 0:1], v_t[:, 0:1], v_t[:, 1:2])
                nc.scalar.activation(out=v_t[:, 2:3], in_=v_t[:, 0:1],
                                     func=mybir.ActivationFunctionType.Sqrt)
                nc.vector.reciprocal(v_t[:, 3:4], v_t[:, 2:3])  # r0 ~ rsqrt
                nc.vector.tensor_tensor(out=v_t[:, 4:5], in0=v_t[:, 3:4], in1=v_t[:, 3:4],
                                        op=mybir.AluOpType.mult)  # r0^2
                nc.vector.tensor_tensor(out=v_t[:, 4:5], in0=v_t[:, 4:5], in1=v_t[:, 0:1],
                                        op=mybir.AluOpType.mult)  # v*r0^2
                nc.vector.tensor_scalar(out=v_t[:, 4:5], in0=v_t[:, 4:5],
                                        scalar1=-0.5, scalar2=1.5,
                                        op0=mybir.AluOpType.mult,
                                        op1=mybir.AluOpType.add)  # 1.5 - 0.5*v*r0^2
                nc.vector.tensor_tensor(out=gst[:, 1:2], in0=v_t[:, 3:4], in1=v_t[:, 4:5],
                                        op=mybir.AluOpType.mult)  # refined rstd

                bc = pmm.tile([128, 2], F32, tag="mm")
                nc.tensor.matmul(bc[:, :], sb_indT[:, :], gst[:, :], start=True, stop=True)

                sv = small.tile([128, 3], F32, tag="sv")
                nc.vector.tensor_tensor(out=sv[:, 0:1], in0=bc[:, 1:2],
                                        in1=sb_gnw[:, ct:ct + 1], op=mybir.AluOpType.mult)
                nc.vector.tensor_tensor(out=sv[:, 1:2], in0=bc[:, 0:1], in1=sv[:, 0:1],
                                        op=mybir.AluOpType.mult)
                nc.vector.tensor_tensor(out=sv[:, 2:3], in0=sb_gnb[:, ct:ct + 1],
                                        in1=sv[:, 1:2], op=mybir.AluOpType.subtract)
                nc.vector.tensor_scalar(out=xn_all[:, ct, :], in0=x_t[:, :],
                                        scalar1=sv[:, 0:1], scalar2=sv[:, 2:3],
                                        op0=mybir.AluOpType.mult,
                                        op1=mybir.AluOpType.add)
                if dbg:
                    dgs = small.tile([128, 3], F32, tag="dgs")
                    nc.vector.tensor_copy(dgs[:, :], sv[:, :])
                    nc.sync.dma_start(d_gs[:, ct, :], dgs[:, :])
                    dxn = xin.tile([128, T], F32, tag="dxn")
                    nc.vector.tensor_copy(dxn[:, :], xn_all[:, ct, :])
                    nc.sync.dma_start(d_xn[128 * ct:128 * (ct + 1), :], dxn[:, :])

            # ---- phase 2: q, k ----
            for p_ in range(2):
                for part in range(2):  # 0=q, 1=k
                    t0 = 0
                    for n in CHUNKS:
                        ps = pmm.tile([128, 512], F32, tag="mm")
                        for kt in range(KT):
                            nc.tensor.matmul(
                                ps[:, 0:n],
                                w_qk[:, kt, 256 * p_ + 128 * part:256 * p_ + 128 * part + 128],
                                xn_all[:, kt, t0:t0 + n],
                                start=(kt == 0), stop=(kt == KT - 1))
                        nc.vector.tensor_scalar_add(
                            qk_all[:, p_, part, t0:t0 + n], ps[:, 0:n],
                            sb_qkb[:, 2 * p_ + part:2 * p_ + part + 1])
                        t0 += n

            if dbg:
                for p_ in range(2):
                    for part in range(2):
                        dq = xin.tile([128, T], F32, tag="dxn")
                        nc.vector.tensor_copy(dq[:, :], qk_all[:, p_, part, :])
                        nc.sync.dma_start(d_qk[:, p_, part, :], dq[:, :])

            # ---- phase 3: vT (v computed transposed) ----
            for st in range(ST):
                ps = pmm.tile([128, 256], F32, tag="mm")
                for kt in range(KT):
                    nc.tensor.matmul(ps[:, :], xn_all[:, kt, 128 * st:128 * (st + 1)],
                                     w_v[:, kt, :], start=(kt == 0), stop=(kt == KT - 1))
                nc.vector.tensor_copy(
                    vT_all[:, st, :, 64:128],
                    ps[:, :].rearrange("p (h c) -> p h c", h=HPC))

            if dbg:
                for st in range(ST):
                    dv = small.tile([128, HPC, 128], F32, tag="dvt")
                    nc.vector.tensor_copy(dv[:, :, :], vT_all[:, st, :, :])
                    nc.sync.dma_start(d_vt[:, st, :, :], dv[:, :, :])

            # ---- phase 4: attention + proj per t-chunk ----
            t0 = 0
            for ci, n in enumerate(CHUNKS):
                a0 = ap_pool.tile([128, 512], F32R, tag="a0")
                a1 = ap_pool.tile([128, 512], F32R, tag="a1")
                a_t = (a0, a0, a1, a1)
                for h in range(HPC):
                    p_, hh = h // 2, h % 2
                    q_sl = qk_all[64 * hh:64 * hh + 64, p_, 0, t0:t0 + n]
                    av = pmm.tile([128, 512], F32, tag="mm")
                    for g in range(NGRP):
                        qs = pqk.tile([128, SG, 512], F32, tag="qk")
                        for i3 in range(SG):
                            st = SG * g + i3
                            nc.tensor.matmul(
                                qs[:, i3, 0:n],
                                qk_all[64 * hh:64 * hh + 64, p_, 1,
                                       128 * st:128 * (st + 1)],
                                q_sl, start=True, stop=True)
                        we = wexpp.tile([128, SG, 512], F32R, tag="we")
                        nc.scalar.activation(out=we[:, :, 0:n], in_=qs[:, :, 0:n],
                                             func=mybir.ActivationFunctionType.Exp)
                        for i3 in range(SG):
                            st = SG * g + i3
                            nc.tensor.matmul(av[:, 0:n], vT_all[:, st, h, :],
                                             we[:, i3, 0:n],
                                             start=(st == 0), stop=(st == ST - 1))
                    rc = rcp.tile([64, 512], F32, tag="rc")
                    nc.vector.reciprocal_approx_fast(out=rc[:, 0:n], in_=av[0:64, 0:n])
                    nc.vector.tensor_tensor(out=a_t[h][64 * hh:64 * hh + 64, 0:n],
                                            in0=av[64:128, 0:n], in1=rc[:, 0:n],
                                            op=mybir.AluOpType.mult)

                for ot in range(CT):
                    pp = pmm.tile([128, 512], F32, tag="mm")
                    nc.tensor.matmul(pp[:, 0:n], w_pj[:, 0, 128 * ot:128 * (ot + 1)],
                                     a0[:, 0:n], start=True, stop=False)
                    nc.tensor.matmul(pp[:, 0:n], w_pj[:, 1, 128 * ot:128 * (ot + 1)],
                                     a1[:, 0:n], start=False, stop=True)
                    t1 = outp.tile([128, 512], F32, tag="t1")
                    nc.vector.tensor_scalar(out=t1[:, 0:n], in0=xn_all[:, ot, t0:t0 + n],
                                            scalar1=sb_rs[:, ot:ot + 1],
                                            scalar2=sb_pb[:, ot:ot + 1],
                                            op0=mybir.AluOpType.mult,
                                            op1=mybir.AluOpType.add)
                    o_t = outp.tile([128, 512], F32, tag="out")
                    nc.vector.tensor_tensor(out=o_t[:, 0:n], in0=pp[:, 0:n],
                                            in1=t1[:, 0:n], op=mybir.AluOpType.add)
                    nc.sync.dma_start(out[128 * ot:128 * (ot + 1), t0:t0 + n],
                                      o_t[:, 0:n])
                t0 += n

    nc.compile()
    return nc


def _shard(inputs):
    x = np.ascontiguousarray(np.asarray(inputs["x"], dtype=np.float32))
    gn_w = np.asarray(inputs["gn_w"], dtype=np.float32)
    gn_b = np.asarray(inputs["gn_b"], dtype=np.float32)
    qkv_w = np.asarray(inputs["qkv_w"], dtype=np.float32)
    qkv_b = np.asarray(inputs["qkv_b"], dtype=np.float32)
    proj_w = np.asarray(inputs["proj_w"], dtype=np.float32)
    proj_b = np.asarray(inputs["proj_b"], dtype=np.float32)

    ind = np.zeros((128, 8), np.float32)
    for c in range(128):
        ind[c, c // GS] = 1.0
    indT = np.ascontiguousarray(ind.T)
    gnw4 = np.ascontiguousarray(gn_w.reshape(CT, 128))
    gnb4 = np.ascontiguousarray(gn_b.reshape(CT, 128))

    in_maps = []
    for core in range(N_CORES):
        b, hg = core // 2, core % 2
        xb = np.ascontiguousarray(x[b].reshape(C, T))

        qkwT = np.empty((C, 512), np.float32)
        qkbm = np.empty((4, 128), np.float32)
        for p_ in range(2):
            for part in range(2):
                for j in range(2):
                    gh = 4 * hg + 2 * p_ + j
                    rows = qkv_w[192 * gh + 64 * part:192 * gh + 64 * part + 64, :]
                    col0 = 256 * p_ + 128 * part + 64 * j
                    qkwT[:, col0:col0 + 64] = (rows * S_ATT).T
                    qkbm[2 * p_ + part, 64 * j:64 * j + 64] = \
                        qkv_b[192 * gh + 64 * part:192 * gh + 64 * part + 64] * S_ATT

        vwT = np.empty((C, 256), np.float32)
        bv = np.empty((256,), np.float32)
        for j in range(HPC):
            gh = 4 * hg + j
            vwT[:, 64 * j:64 * j + 64] = qkv_w[192 * gh + 128:192 * gh + 192, :].T
            bv[64 * j:64 * j + 64] = qkv_b[192 * gh + 128:192 * gh + 192]

        pw = proj_w[:, 256 * hg:256 * hg + 256]
        projwT = np.ascontiguousarray(pw.T)
        pb = pw @ bv
        if hg == 0:
            pb = pb + proj_b
        rs = np.full((CT, 128), 1.0 if hg == 0 else 0.0, np.float32)

        in_maps.append({
            "xb": xb,
            "qkwT": np.ascontiguousarray(qkwT),
            "vwT": np.ascontiguousarray(vwT),
            "projwT": projwT,
            "qkb": np.ascontiguousarray(qkbm),
            "gnw": gnw4, "gnb": gnb4,
            "ind": ind, "indT": indT,
            "resscale": rs,
            "pbeff": np.ascontiguousarray(pb.reshape(CT, 128).astype(np.float32)),
        })
    return in_maps


def _ensure_ntff_hook():
    """Install the antenv.axon_hooks shim so BASS_TRACE=1 can capture NTFF
    profiles through libaxon_pjrt.so (the image ships the .so but not the
    python-side hook module)."""
    try:
        from antenv.axon_hooks import get_axon_ntff_profile_hook  # noqa: F401
        return
    except ImportError:
        pass
    import types

    try:
        import antenv
        from trn_agent_boot.trn_boot import _ntff_profile_via_ctypes
    except ImportError:
        return
    mod = types.ModuleType("antenv.axon_hooks")
    holder = {"hook": None}
    mod.set_axon_ntff_profile_hook = lambda h: holder.__setitem__("hook", h)
    mod.get_axon_ntff_profile_hook = lambda: holder["hook"]
    sys.modules["antenv.axon_hooks"] = mod
    antenv.axon_hooks = mod
    so = "/opt/axon/libaxon_pjrt.so"
    if os.path.exists(so):
        hook = _ntff_profile_via_ctypes(so)
        if hook is not None:
            mod.set_axon_ntff_profile_hook(hook)


LAST_RESULTS = None


def kernel(**inputs):
    global LAST_RESULTS
    if os.environ.get("BASS_TRACE"):
        _ensure_ntff_hook()
    if "nc" not in _CACHE:
        _CACHE["nc"] = _build()
    nc = _CACHE["nc"]
    in_maps = _shard(inputs)
    res = run_bass_kernel_spmd(nc, in_maps, core_ids=list(range(N_CORES)))
    LAST_RESULTS = res
    out = np.empty((B, C, T), np.float32)
    for b in range(B):
        out[b] = res.results[2 * b]["out"] + res.results[2 * b + 1]["out"]
    return out.reshape(B, C, 48, 48)


# revision 8
# speedup vs baseline: 1.1423x; 1.0392x over previous
import math
import os
import sys

sys.path.insert(0, "/opt/trn_rl_repo")

import numpy as np

import concourse.bass as bass
import concourse.tile as tile
from concourse import bacc, mybir
from concourse.bass_utils import run_bass_kernel_spmd

# Problem constants (nn_AttentionBlock: B=4, C=512, H=W=48, 8 heads, GN32)
B = 4
C = 512
T = 2304  # 48*48
NH = 8
CH = 64  # C // NH
NG = 32  # groupnorm groups
GS = 16  # channels per group
EPS = 1e-5
S_ATT = 1.0 / math.sqrt(math.sqrt(CH))

N_CORES = 8
HPC = 4  # heads per core
CT = 4  # channel tiles of 128
KT = 4  # contraction tiles of 128 over C
ST = 18  # sequence tiles of 128 over T
SG = 3  # s-tiles per exp group
NGRP = ST // SG
CHUNKS = [512, 512, 512, 512, 256]  # t chunks (sum = 2304)

F32 = mybir.dt.float32
F32R = mybir.dt.float32r
BF16 = mybir.dt.bfloat16

_CACHE = {}


def _build():
    nc = bacc.Bacc("TRN2", target_bir_lowering=False, debug=False,
                   enable_asserts=True, num_devices=N_CORES)

    xb = nc.dram_tensor("xb", [C, T], F32, kind="ExternalInput").ap()
    qkwT = nc.dram_tensor("qkwT", [C, 512], F32R, kind="ExternalInput").ap()
    vwT = nc.dram_tensor("vwT", [C, 256], F32R, kind="ExternalInput").ap()
    projwT = nc.dram_tensor("projwT", [256, C], F32R, kind="ExternalInput").ap()
    qkb = nc.dram_tensor("qkb", [4, 128], F32, kind="ExternalInput").ap()
    gnw = nc.dram_tensor("gnw", [CT, 128], F32, kind="ExternalInput").ap()
    gnb = nc.dram_tensor("gnb", [CT, 128], F32, kind="ExternalInput").ap()
    ind = nc.dram_tensor("ind", [128, 8], F32, kind="ExternalInput").ap()
    indT = nc.dram_tensor("indT", [8, 128], F32, kind="ExternalInput").ap()
    resscale = nc.dram_tensor("resscale", [CT, 128], F32, kind="ExternalInput").ap()
    pbeff = nc.dram_tensor("pbeff", [CT, 128], F32, kind="ExternalInput").ap()
    out = nc.dram_tensor("out", [C, T], F32, kind="ExternalOutput").ap()
    dbg = os.environ.get("KBG_DEBUG")
    if dbg:
        d_xn = nc.dram_tensor("d_xn", [CT * 128, T], F32, kind="ExternalOutput").ap()
        d_qk = nc.dram_tensor("d_qk", [128, 2, 2, T], F32, kind="ExternalOutput").ap()
        d_vt = nc.dram_tensor("d_vt", [128, ST, HPC, 128], F32, kind="ExternalOutput").ap()
        d_gs = nc.dram_tensor("d_gs", [128, CT, 3], F32, kind="ExternalOutput").ap()

    with tile.TileContext(nc) as tc:
        with (
            tc.tile_pool(name="persist", bufs=1) as persist,
            tc.tile_pool(name="xin", bufs=2) as xin,
            tc.tile_pool(name="wexp", bufs=5) as wexpp,
            tc.tile_pool(name="small", bufs=4) as small,
            tc.tile_pool(name="ap", bufs=2) as ap_pool,
            tc.tile_pool(name="outp", bufs=2) as outp,
            tc.tile_pool(name="rcp", bufs=2) as rcp,
            tc.tile_pool(name="pqk", bufs=2, space="PSUM") as pqk,
            tc.tile_pool(name="pmm", bufs=2, space="PSUM") as pmm,
        ):
            # ---- persistent SBUF state ----
            xn_all = persist.tile([128, CT, T], F32R)        # normalized x
            qk_all = persist.tile([128, 2, 2, T], BF16)      # [qA|qB / kA|kB] per pair
            vT_all = persist.tile([128, ST, HPC, 128], F32R)  # [vT(64) | ones(64)] per head
            w_qk = persist.tile([128, KT, 512], F32R)
            w_v = persist.tile([128, KT, 256], F32R)
            w_pj = persist.tile([128, 2, 512], F32R)
            sb_qkb = persist.tile([128, 4], F32)
            sb_gnw = persist.tile([128, CT], F32)
            sb_gnb = persist.tile([128, CT], F32)
            sb_rs = persist.tile([128, CT], F32)
            sb_pb = persist.tile([128, CT], F32)
            sb_ind = persist.tile([128, 8], F32)
            sb_indT = persist.tile([8, 128], F32)

            # weight / constant loads
            for kt in range(KT):
                nc.sync.dma_start(w_qk[:, kt, :], qkwT[128 * kt:128 * (kt + 1), :])
                nc.sync.dma_start(w_v[:, kt, :], vwT[128 * kt:128 * (kt + 1), :])
            for j in range(2):
                nc.sync.dma_start(w_pj[:, j, :], projwT[128 * j:128 * (j + 1), :])
            # [4,128] dram -> [128,4] sbuf (transpose via AP)
            for (dst, src) in ((sb_qkb, qkb), (sb_gnw, gnw), (sb_gnb, gnb),
                               (sb_rs, resscale), (sb_pb, pbeff)):
                nc.sync.dma_start(dst[:, :], src.transpose([1, 0]))
            nc.sync.dma_start(sb_ind[:, :], ind[:, :])
            nc.sync.dma_start(sb_indT[:, :], indT[:, :])
            # ones columns of vT_all
            nc.vector.memset(vT_all[:, :, :, 0:64].bitcast(F32), 1.0)

            # ---- phase 1: groupnorm -> xn_all ----
            for ct in range(CT):
                x_t = xin.tile([128, T], F32, tag="x")
                nc.sync.dma_start(x_t[:, :], xb[128 * ct:128 * (ct + 1), :])

                stats = small.tile([128, 9, 6], F32, tag="stats")
                for j in range(9):
                    nc.vector.bn_stats(out=stats[:, j, :], in_=x_t[:, 256 * j:256 * (j + 1)])
                mv = small.tile([128, 2], F32, tag="mv")
                nc.vector.bn_aggr(out=mv[:, :], in_=stats[:, :, :])

                em2 = small.tile([128, 2], F32, tag="em2")
                nc.vector.tensor_copy(em2[:, 0:1], mv[:, 0:1])
                nc.vector.tensor_tensor(out=em2[:, 1:2], in0=mv[:, 0:1], in1=mv[:, 0:1],
                                        op=mybir.AluOpType.mult)
                nc.vector.tensor_add(em2[:, 1:2], em2[:, 1:2], mv[:, 1:2])

                gsum = pmm.tile([8, 2], F32, tag="mm")
                nc.tensor.matmul(gsum[:, :], sb_ind[:, :], em2[:, :], start=True, stop=True)

                # group stats -> [mu, rstd] with a Newton-refined rsqrt
                gst = small.tile([8, 2], F32, tag="gst")
                v_t = small.tile([8, 5], F32, tag="gtmp")
                nc.vector.tensor_scalar_mul(gst[:, 0:1], gsum[:, 0:1], 1.0 / GS)
                nc.vector.tensor_scalar(out=v_t[:, 0:1], in0=gsum[:, 1:2],
                                        scalar1=1.0 / GS, scalar2=EPS,
                                        op0=mybir.AluOpType.mult,
                                        op1=mybir.AluOpType.add)
                # v holds E[x^2]+eps; subtract mu^2 -> var+eps
                nc.vector.tensor_tensor(out=v_t[:, 1:2], in0=gst[:, 0:1], in1=gst[:, 0:1],
                                        op=mybir.AluOpType.mult)
                nc.vector.tensor_sub(v_t[:, 0:1], v_t[:, 0:1], v_t[:, 1:2])
                nc.scalar.activation(out=v_t[:, 2:3], in_=v_t[:, 0:1],
                                     func=mybir.ActivationFunctionType.Sqrt)
                nc.vector.reciprocal(v_t[:, 3:4], v_t[:, 2:3])  # r0 ~ rsqrt
                nc.vector.tensor_tensor(out=v_t[:, 4:5], in0=v_t[:, 3:4], in1=v_t[:, 3:4],
                                        op=mybir.AluOpType.mult)  # r0^2
                nc.vector.tensor_tensor(out=v_t[:, 4:5], in0=v_t[:, 4:5], in1=v_t[:, 0:1],
                                        op=mybir.AluOpType.mult)  # v*r0^2
                nc.vector.tensor_scalar(out=v_t[:, 4:5], in0=v_t[:, 4:5],
                                        scalar1=-0.5, scalar2=1.5,
                                        op0=mybir.AluOpType.mult,
                                        op1=mybir.AluOpType.add)  # 1.5 - 0.5*v*r0^2
                nc.vector.tensor_tensor(out=gst[:, 1:2], in0=v_t[:, 3:4], in1=v_t[:, 4:5],
                                        op=mybir.AluOpType.mult)  # refined rstd

                bc = pmm.tile([128, 2], F32, tag="mm")
                nc.tensor.matmul(bc[:, :], sb_indT[:, :], gst[:, :], start=True, stop=True)

                sv = small.tile([128, 3], F32, tag="sv")
                nc.vector.tensor_tensor(out=sv[:, 0:1], in0=bc[:, 1:2],
                                        in1=sb_gnw[:, ct:ct + 1], op=mybir.AluOpType.mult)
                nc.vector.tensor_tensor(out=sv[:, 1:2], in0=bc[:, 0:1], in1=sv[:, 0:1],
                                        op=mybir.AluOpType.mult)
                nc.vector.tensor_tensor(out=sv[:, 2:3], in0=sb_gnb[:, ct:ct + 1],
                                        in1=sv[:, 1:2], op=mybir.AluOpType.subtract)
                nc.vector.tensor_scalar(out=xn_all[:, ct, :], in0=x_t[:, :],
                                        scalar1=sv[:, 0:1], scalar2=sv[:, 2:3],
                                        op0=mybir.AluOpType.mult,
                                        op1=mybir.AluOpType.add)
                if dbg:
                    dgs = small.tile([128, 3], F32, tag="dgs")
                    nc.vector.tensor_copy(dgs[:, :], sv[:, :])
                    nc.sync.dma_start(d_gs[:, ct, :], dgs[:, :])
                    dxn = xin.tile([128, T], F32, tag="dxn")
                    nc.vector.tensor_copy(dxn[:, :], xn_all[:, ct, :])
                    nc.sync.dma_start(d_xn[128 * ct:128 * (ct + 1), :], dxn[:, :])

            # ---- phase 2: q, k for pair 0 ----
            def emit_qk(p_):
                for part in range(2):  # 0=q, 1=k
                    t0 = 0
                    for n in CHUNKS:
                        ps = pmm.tile([128, 512], F32, tag="mm")
                        for kt in range(KT):
                            nc.tensor.matmul(
                                ps[:, 0:n],
                                w_qk[:, kt, 256 * p_ + 128 * part:256 * p_ + 128 * part + 128],
                                xn_all[:, kt, t0:t0 + n],
                                start=(kt == 0), stop=(kt == KT - 1))
                        nc.vector.tensor_scalar_add(
                            qk_all[:, p_, part, t0:t0 + n], ps[:, 0:n],
                            sb_qkb[:, 2 * p_ + part:2 * p_ + part + 1])
                        t0 += n

            emit_qk(0)

            # ---- phase 3: vT (v computed transposed) ----
            for st in range(ST):
                ps = pmm.tile([128, 256], F32, tag="mm")
                for kt in range(KT):
                    nc.tensor.matmul(ps[:, :], xn_all[:, kt, 128 * st:128 * (st + 1)],
                                     w_v[:, kt, :], start=(kt == 0), stop=(kt == KT - 1))
                nc.vector.tensor_copy(
                    vT_all[:, st, :, 64:128],
                    ps[:, :].rearrange("p (h c) -> p h c", h=HPC))

            # ---- phase 4: attention, pair-major; pair1 qk + proj act as PE filler ----
            a_all = persist.tile([128, 2, T], F32R)   # ac-tile p_: heads 2p_,2p_+1

            def emit_attention(p_):
                t0 = 0
                for ci, n in enumerate(CHUNKS):
                    avs = []
                    for hh in range(2):
                        h = 2 * p_ + hh
                        q_sl = qk_all[64 * hh:64 * hh + 64, p_, 0, t0:t0 + n]
                        av = pmm.tile([128, 512], F32, tag="mm")
                        for g in range(NGRP):
                            qs = pqk.tile([128, SG, 512], F32, tag="qk")
                            for i3 in range(SG):
                                st = SG * g + i3
                                nc.tensor.matmul(
                                    qs[:, i3, 0:n],
                                    qk_all[64 * hh:64 * hh + 64, p_, 1,
                                           128 * st:128 * (st + 1)],
                                    q_sl, start=True, stop=True)
                            we = wexpp.tile([128, SG, 512], F32R, tag="we")
                            nc.scalar.activation(out=we[:, :, 0:n], in_=qs[:, :, 0:n],
                                                 func=mybir.ActivationFunctionType.Exp)
                            for i3 in range(SG):
                                st = SG * g + i3
                                nc.tensor.matmul(av[:, 0:n], vT_all[:, st, h, :],
                                                 we[:, i3, 0:n],
                                                 start=(st == 0), stop=(st == ST - 1))
                        avs.append(av)
                    for hh in range(2):
                        av = avs[hh]
                        rc = rcp.tile([64, 512], F32, tag="rc")
                        nc.vector.reciprocal_approx_fast(out=rc[:, 0:n], in_=av[0:64, 0:n])
                        nc.vector.tensor_tensor(out=a_all[64 * hh:64 * hh + 64, p_, t0:t0 + n],
                                                in0=av[64:128, 0:n], in1=rc[:, 0:n],
                                                op=mybir.AluOpType.mult)
                    t0 += n

            emit_attention(0)
            emit_qk(1)       # traced after pair0 attention -> PE filler
            emit_attention(1)

            # ---- phase 5: proj + residual ----
            t0 = 0
            for ci, n in enumerate(CHUNKS):
                for ot in range(CT):
                    pp = pmm.tile([128, 512], F32, tag="mm")
                    nc.tensor.matmul(pp[:, 0:n], w_pj[:, 0, 128 * ot:128 * (ot + 1)],
                                     a_all[:, 0, t0:t0 + n], start=True, stop=False)
                    nc.tensor.matmul(pp[:, 0:n], w_pj[:, 1, 128 * ot:128 * (ot + 1)],
                                     a_all[:, 1, t0:t0 + n], start=False, stop=True)
                    t1 = outp.tile([128, 512], F32, tag="t1")
                    nc.vector.tensor_scalar(out=t1[:, 0:n], in0=xn_all[:, ot, t0:t0 + n],
                                            scalar1=sb_rs[:, ot:ot + 1],
                                            scalar2=sb_pb[:, ot:ot + 1],
                                            op0=mybir.AluOpType.mult,
                                            op1=mybir.AluOpType.add)
                    o_t = outp.tile([128, 512], F32, tag="out")
                    nc.vector.tensor_tensor(out=o_t[:, 0:n], in0=pp[:, 0:n],
                                            in1=t1[:, 0:n], op=mybir.AluOpType.add)
                    nc.sync.dma_start(out[128 * ot:128 * (ot + 1), t0:t0 + n],
                                      o_t[:, 0:n])
                t0 += n

    nc.compile()
    return nc


def _shard(inputs):
    x = np.ascontiguousarray(np.asarray(inputs["x"], dtype=np.float32))
    gn_w = np.asarray(inputs["gn_w"], dtype=np.float32)
    gn_b = np.asarray(inputs["gn_b"], dtype=np.float32)
    qkv_w = np.asarray(inputs["qkv_w"], dtype=np.float32)
    qkv_b = np.asarray(inputs["qkv_b"], dtype=np.float32)
    proj_w = np.asarray(inputs["proj_w"], dtype=np.float32)
    proj_b = np.asarray(inputs["proj_b"], dtype=np.float32)

    ind = np.zeros((128, 8), np.float32)
    for c in range(128):
        ind[c, c // GS] = 1.0
    indT = np.ascontiguousarray(ind.T)
    gnw4 = np.ascontiguousarray(gn_w.reshape(CT, 128))
    gnb4 = np.ascontiguousarray(gn_b.reshape(CT, 128))

    in_maps = []
    for core in range(N_CORES):
        b, hg = core // 2, core % 2
        xb = np.ascontiguousarray(x[b].reshape(C, T))

        qkwT = np.empty((C, 512), np.float32)
        qkbm = np.empty((4, 128), np.float32)
        for p_ in range(2):
            for part in range(2):
                for j in range(2):
                    gh = 4 * hg + 2 * p_ + j
                    rows = qkv_w[192 * gh + 64 * part:192 * gh + 64 * part + 64, :]
                    col0 = 256 * p_ + 128 * part + 64 * j
                    qkwT[:, col0:col0 + 64] = (rows * S_ATT).T
                    qkbm[2 * p_ + part, 64 * j:64 * j + 64] = \
                        qkv_b[192 * gh + 64 * part:192 * gh + 64 * part + 64] * S_ATT

        vwT = np.empty((C, 256), np.float32)
        bv = np.empty((256,), np.float32)
        for j in range(HPC):
            gh = 4 * hg + j
            vwT[:, 64 * j:64 * j + 64] = qkv_w[192 * gh + 128:192 * gh + 192, :].T
            bv[64 * j:64 * j + 64] = qkv_b[192 * gh + 128:192 * gh + 192]

        pw = proj_w[:, 256 * hg:256 * hg + 256]
        projwT = np.ascontiguousarray(pw.T)
        pb = pw @ bv
        if hg == 0:
            pb = pb + proj_b
        rs = np.full((CT, 128), 1.0 if hg == 0 else 0.0, np.float32)

        in_maps.append({
            "xb": xb,
            "qkwT": np.ascontiguousarray(qkwT),
            "vwT": np.ascontiguousarray(vwT),
            "projwT": projwT,
            "qkb": np.ascontiguousarray(qkbm),
            "gnw": gnw4, "gnb": gnb4,
            "ind": ind, "indT": indT,
            "resscale": rs,
            "pbeff": np.ascontiguousarray(pb.reshape(CT, 128).astype(np.float32)),
        })
    return in_maps


def _ensure_ntff_hook():
    """Install the antenv.axon_hooks shim so BASS_TRACE=1 can capture NTFF
    profiles through libaxon_pjrt.so (the image ships the .so but not the
    python-side hook module)."""
    try:
        from antenv.axon_hooks import get_axon_ntff_profile_hook  # noqa: F401
        return
    except ImportError:
        pass
    import types

    try:
        import antenv
        from trn_agent_boot.trn_boot import _ntff_profile_via_ctypes
    except ImportError:
        return
    mod = types.ModuleType("antenv.axon_hooks")
    holder = {"hook": None}
    mod.set_axon_ntff_profile_hook = lambda h: holder.__setitem__("hook", h)
    mod.get_axon_ntff_profile_hook = lambda: holder["hook"]
    sys.modules["antenv.axon_hooks"] = mod
    antenv.axon_hooks = mod
    so = "/opt/axon/libaxon_pjrt.so"
    if os.path.exists(so):
        hook = _ntff_profile_via_ctypes(so)
        if hook is not None:
            mod.set_axon_ntff_profile_hook(hook)


LAST_RESULTS = None


def kernel(**inputs):
    global LAST_RESULTS
    if os.environ.get("BASS_TRACE"):
        _ensure_ntff_hook()
    if "nc" not in _CACHE:
        _CACHE["nc"] = _build()
    nc = _CACHE["nc"]
    in_maps = _shard(inputs)
    res = run_bass_kernel_spmd(nc, in_maps, core_ids=list(range(N_CORES)))
    LAST_RESULTS = res
    out = np.empty((B, C, T), np.float32)
    for b in range(B):
        out[b] = res.results[2 * b]["out"] + res.results[2 * b + 1]["out"]
    return out.reshape(B, C, 48, 48)


# revision 10
# speedup vs baseline: 1.1798x; 1.0328x over previous
import math
import os
import sys

sys.path.insert(0, "/opt/trn_rl_repo")

import numpy as np

import concourse.bass as bass
import concourse.tile as tile
from concourse import bacc, mybir
from concourse.bass_utils import run_bass_kernel_spmd

# Problem constants (nn_AttentionBlock: B=4, C=512, H=W=48, 8 heads, GN32)
B = 4
C = 512
T = 2304  # 48*48
NH = 8
CH = 64  # C // NH
NG = 32  # groupnorm groups
GS = 16  # channels per group
EPS = 1e-5
S_ATT = 1.0 / math.sqrt(math.sqrt(CH))

N_CORES = 8
HPC = 4  # heads per core
CT = 4  # channel tiles of 128
KT = 4  # contraction tiles of 128 over C
ST = 18  # sequence tiles of 128 over T
SG = 3  # s-tiles per exp group
NGRP = ST // SG
CHUNKS = [512, 512, 512, 512, 256]  # t chunks (sum = 2304)

F32 = mybir.dt.float32
F32R = mybir.dt.float32r
BF16 = mybir.dt.bfloat16

_CACHE = {}


def _build():
    nc = bacc.Bacc("TRN2", target_bir_lowering=False, debug=False,
                   enable_asserts=True, num_devices=N_CORES)

    xb = nc.dram_tensor("xb", [C, T], F32, kind="ExternalInput").ap()
    qkwT = nc.dram_tensor("qkwT", [C, 512], F32R, kind="ExternalInput").ap()
    vwT = nc.dram_tensor("vwT", [C, 256], F32R, kind="ExternalInput").ap()
    projwT = nc.dram_tensor("projwT", [256, C], F32R, kind="ExternalInput").ap()
    qkb = nc.dram_tensor("qkb", [4, 128], F32, kind="ExternalInput").ap()
    gnw = nc.dram_tensor("gnw", [CT, 128], F32, kind="ExternalInput").ap()
    gnb = nc.dram_tensor("gnb", [CT, 128], F32, kind="ExternalInput").ap()
    ind = nc.dram_tensor("ind", [128, 8], F32, kind="ExternalInput").ap()
    indT = nc.dram_tensor("indT", [8, 128], F32, kind="ExternalInput").ap()
    resscale = nc.dram_tensor("resscale", [CT, 128], F32, kind="ExternalInput").ap()
    pbeff = nc.dram_tensor("pbeff", [CT, 128], F32, kind="ExternalInput").ap()
    out = nc.dram_tensor("out", [C, T], F32, kind="ExternalOutput").ap()
    dbg = os.environ.get("KBG_DEBUG")
    if dbg:
        d_xn = nc.dram_tensor("d_xn", [CT * 128, T], F32, kind="ExternalOutput").ap()
        d_qk = nc.dram_tensor("d_qk", [128, 2, 2, T], F32, kind="ExternalOutput").ap()
        d_vt = nc.dram_tensor("d_vt", [128, ST, HPC, 128], F32, kind="ExternalOutput").ap()
        d_gs = nc.dram_tensor("d_gs", [128, CT, 3], F32, kind="ExternalOutput").ap()

    with tile.TileContext(nc) as tc:
        with (
            tc.tile_pool(name="persist", bufs=1) as persist,
            tc.tile_pool(name="xin", bufs=2) as xin,
            tc.tile_pool(name="wexp", bufs=5) as wexpp,
            tc.tile_pool(name="small", bufs=4) as small,
            tc.tile_pool(name="ap", bufs=2) as ap_pool,
            tc.tile_pool(name="outp", bufs=2) as outp,
            tc.tile_pool(name="rcp", bufs=2) as rcp,
            tc.tile_pool(name="pqk", bufs=2, space="PSUM") as pqk,
            tc.tile_pool(name="pmm", bufs=2, space="PSUM") as pmm,
        ):
            # ---- persistent SBUF state ----
            xn_all = persist.tile([128, CT, T], F32R)        # normalized x
            qk_all = persist.tile([128, 2, 2, T], BF16)      # [qA|qB / kA|kB] per pair
            vT_all = persist.tile([128, ST, HPC, 128], F32R)  # [vT(64) | ones(64)] per head
            w_qk = persist.tile([128, KT, 512], F32R)
            w_v = persist.tile([128, KT, 256], F32R)
            w_pj = persist.tile([128, 2, 512], F32R)
            sb_qkb = persist.tile([128, 4], F32)
            sb_gnw = persist.tile([128, CT], F32)
            sb_gnb = persist.tile([128, CT], F32)
            sb_rs = persist.tile([128, CT], F32)
            sb_pb = persist.tile([128, CT], F32)
            sb_ind = persist.tile([128, 8], F32)
            sb_indT = persist.tile([8, 128], F32)

            # weight / constant loads
            for kt in range(KT):
                nc.sync.dma_start(w_qk[:, kt, :], qkwT[128 * kt:128 * (kt + 1), :])
                nc.sync.dma_start(w_v[:, kt, :], vwT[128 * kt:128 * (kt + 1), :])
            for j in range(2):
                nc.sync.dma_start(w_pj[:, j, :], projwT[128 * j:128 * (j + 1), :])
            # [4,128] dram -> [128,4] sbuf (transpose via AP)
            for (dst, src) in ((sb_qkb, qkb), (sb_gnw, gnw), (sb_gnb, gnb),
                               (sb_rs, resscale), (sb_pb, pbeff)):
                nc.sync.dma_start(dst[:, :], src.transpose([1, 0]))
            nc.sync.dma_start(sb_ind[:, :], ind[:, :])
            nc.sync.dma_start(sb_indT[:, :], indT[:, :])
            # ones columns of vT_all
            nc.vector.memset(vT_all[:, :, :, 0:64].bitcast(F32), 1.0)

            # ---- phase 1: groupnorm -> xn_all ----
            for ct in range(CT):
                x_t = xin.tile([128, T], F32, tag="x")
                nc.sync.dma_start(x_t[:, :], xb[128 * ct:128 * (ct + 1), :])

                stats = small.tile([128, 9, 6], F32, tag="stats")
                for j in range(9):
                    nc.vector.bn_stats(out=stats[:, j, :], in_=x_t[:, 256 * j:256 * (j + 1)])
                mv = small.tile([128, 2], F32, tag="mv")
                nc.vector.bn_aggr(out=mv[:, :], in_=stats[:, :, :])

                em2 = small.tile([128, 2], F32, tag="em2")
                nc.vector.tensor_copy(em2[:, 0:1], mv[:, 0:1])
                nc.vector.tensor_tensor(out=em2[:, 1:2], in0=mv[:, 0:1], in1=mv[:, 0:1],
                                        op=mybir.AluOpType.mult)
                nc.vector.tensor_add(em2[:, 1:2], em2[:, 1:2], mv[:, 1:2])

                gsum = pmm.tile([8, 2], F32, tag="mm")
                nc.tensor.matmul(gsum[:, :], sb_ind[:, :], em2[:, :], start=True, stop=True)

                # group stats -> [mu, rstd] with a Newton-refined rsqrt
                gst = small.tile([8, 2], F32, tag="gst")
                v_t = small.tile([8, 5], F32, tag="gtmp")
                nc.vector.tensor_scalar_mul(gst[:, 0:1], gsum[:, 0:1], 1.0 / GS)
                nc.vector.tensor_scalar(out=v_t[:, 0:1], in0=gsum[:, 1:2],
                                        scalar1=1.0 / GS, scalar2=EPS,
                                        op0=mybir.AluOpType.mult,
                                        op1=mybir.AluOpType.add)
                # v holds E[x^2]+eps; subtract mu^2 -> var+eps
                nc.vector.tensor_tensor(out=v_t[:, 1:2], in0=gst[:, 0:1], in1=gst[:, 0:1],
                                        op=mybir.AluOpType.mult)
                nc.vector.tensor_sub(v_t[:, 0:1], v_t[:, 0:1], v_t[:, 1:2])
                nc.scalar.activation(out=v_t[:, 2:3], in_=v_t[:, 0:1],
                                     func=mybir.ActivationFunctionType.Sqrt)
                nc.vector.reciprocal(v_t[:, 3:4], v_t[:, 2:3])  # r0 ~ rsqrt
                nc.vector.tensor_tensor(out=v_t[:, 4:5], in0=v_t[:, 3:4], in1=v_t[:, 3:4],
                                        op=mybir.AluOpType.mult)  # r0^2
                nc.vector.tensor_tensor(out=v_t[:, 4:5], in0=v_t[:, 4:5], in1=v_t[:, 0:1],
                                        op=mybir.AluOpType.mult)  # v*r0^2
                nc.vector.tensor_scalar(out=v_t[:, 4:5], in0=v_t[:, 4:5],
                                        scalar1=-0.5, scalar2=1.5,
                                        op0=mybir.AluOpType.mult,
                                        op1=mybir.AluOpType.add)  # 1.5 - 0.5*v*r0^2
                nc.vector.tensor_tensor(out=gst[:, 1:2], in0=v_t[:, 3:4], in1=v_t[:, 4:5],
                                        op=mybir.AluOpType.mult)  # refined rstd

                bc = pmm.tile([128, 2], F32, tag="mm")
                nc.tensor.matmul(bc[:, :], sb_indT[:, :], gst[:, :], start=True, stop=True)

                sv = small.tile([128, 3], F32, tag="sv")
                nc.vector.tensor_tensor(out=sv[:, 0:1], in0=bc[:, 1:2],
                                        in1=sb_gnw[:, ct:ct + 1], op=mybir.AluOpType.mult)
                nc.vector.tensor_tensor(out=sv[:, 1:2], in0=bc[:, 0:1], in1=sv[:, 0:1],
                                        op=mybir.AluOpType.mult)
                nc.vector.tensor_tensor(out=sv[:, 2:3], in0=sb_gnb[:, ct:ct + 1],
                                        in1=sv[:, 1:2], op=mybir.AluOpType.subtract)
                nc.vector.tensor_scalar(out=xn_all[:, ct, :], in0=x_t[:, :],
                                        scalar1=sv[:, 0:1], scalar2=sv[:, 2:3],
                                        op0=mybir.AluOpType.mult,
                                        op1=mybir.AluOpType.add)
                if dbg:
                    dgs = small.tile([128, 3], F32, tag="dgs")
                    nc.vector.tensor_copy(dgs[:, :], sv[:, :])
                    nc.sync.dma_start(d_gs[:, ct, :], dgs[:, :])
                    dxn = xin.tile([128, T], F32, tag="dxn")
                    nc.vector.tensor_copy(dxn[:, :], xn_all[:, ct, :])
                    nc.sync.dma_start(d_xn[128 * ct:128 * (ct + 1), :], dxn[:, :])

            # ---- phase 2: q, k for pair 0 ----
            def emit_qk(p_):
                for part in range(2):  # 0=q, 1=k
                    t0 = 0
                    for n in CHUNKS:
                        ps = pmm.tile([128, 512], F32, tag="mm")
                        for kt in range(KT):
                            nc.tensor.matmul(
                                ps[:, 0:n],
                                w_qk[:, kt, 256 * p_ + 128 * part:256 * p_ + 128 * part + 128],
                                xn_all[:, kt, t0:t0 + n],
                                start=(kt == 0), stop=(kt == KT - 1))
                        nc.vector.tensor_scalar_add(
                            qk_all[:, p_, part, t0:t0 + n], ps[:, 0:n],
                            sb_qkb[:, 2 * p_ + part:2 * p_ + part + 1])
                        t0 += n

            emit_qk(0)

            # ---- phase 3: vT (v computed transposed) ----
            for st in range(ST):
                ps = pmm.tile([128, 256], F32, tag="mm")
                for kt in range(KT):
                    nc.tensor.matmul(ps[:, :], xn_all[:, kt, 128 * st:128 * (st + 1)],
                                     w_v[:, kt, :], start=(kt == 0), stop=(kt == KT - 1))
                nc.vector.tensor_copy(
                    vT_all[:, st, :, 64:128],
                    ps[:, :].rearrange("p (h c) -> p h c", h=HPC))

            # ---- phase 4: attention, pair-major; pair1 qk + proj act as PE filler ----
            a_all = persist.tile([128, 2, T], F32R)   # ac-tile p_: heads 2p_,2p_+1

            def emit_attention(p_, tail=None):
                t0 = 0
                for ci, n in enumerate(CHUNKS):
                    # both heads of the pair interleaved: QK matmuls touch
                    # disjoint PE row halves (partitions 0-63 vs 64-127) and
                    # different PSUM banks, so they execute concurrently
                    avs = [pmm.tile([128, 512], F32, tag="mm", name=f"av{hh}")
                           for hh in range(2)]
                    for g in range(NGRP):
                        qss = [pqk.tile([128, SG, 512], F32, tag="qk", name=f"qs{hh}")
                               for hh in range(2)]
                        for i3 in range(SG):
                            st = SG * g + i3
                            for hh in range(2):
                                nc.tensor.matmul(
                                    qss[hh][:, i3, 0:n],
                                    qk_all[64 * hh:64 * hh + 64, p_, 1,
                                           128 * st:128 * (st + 1)],
                                    qk_all[64 * hh:64 * hh + 64, p_, 0, t0:t0 + n],
                                    start=True, stop=True)
                        wes = []
                        for hh in range(2):
                            we = wexpp.tile([128, SG, 512], F32R, tag="we")
                            nc.scalar.activation(out=we[:, :, 0:n], in_=qss[hh][:, :, 0:n],
                                                 func=mybir.ActivationFunctionType.Exp)
                            wes.append(we)
                        for hh in range(2):
                            h = 2 * p_ + hh
                            for i3 in range(SG):
                                st = SG * g + i3
                                nc.tensor.matmul(avs[hh][:, 0:n], vT_all[:, st, h, :],
                                                 wes[hh][:, i3, 0:n],
                                                 start=(st == 0), stop=(st == ST - 1))
                    for hh in range(2):
                        av = avs[hh]
                        rc = rcp.tile([64, 512], F32, tag="rc")
                        nc.vector.reciprocal_approx_fast(out=rc[:, 0:n], in_=av[0:64, 0:n])
                        nc.vector.tensor_tensor(out=a_all[64 * hh:64 * hh + 64, p_, t0:t0 + n],
                                                in0=av[64:128, 0:n], in1=rc[:, 0:n],
                                                op=mybir.AluOpType.mult)
                    if tail is not None:
                        tail(ci, t0, n)
                    t0 += n

            def emit_proj(ci, t0, n):
                for ot in range(CT):
                    pp = pmm.tile([128, 512], F32, tag="mm")
                    nc.tensor.matmul(pp[:, 0:n], w_pj[:, 0, 128 * ot:128 * (ot + 1)],
                                     a_all[:, 0, t0:t0 + n], start=True, stop=False)
                    nc.tensor.matmul(pp[:, 0:n], w_pj[:, 1, 128 * ot:128 * (ot + 1)],
                                     a_all[:, 1, t0:t0 + n], start=False, stop=True)
                    t1 = outp.tile([128, 512], F32, tag="t1")
                    nc.vector.tensor_scalar(out=t1[:, 0:n], in0=xn_all[:, ot, t0:t0 + n],
                                            scalar1=sb_rs[:, ot:ot + 1],
                                            scalar2=sb_pb[:, ot:ot + 1],
                                            op0=mybir.AluOpType.mult,
                                            op1=mybir.AluOpType.add)
                    o_t = outp.tile([128, 512], F32, tag="out")
                    nc.vector.tensor_tensor(out=o_t[:, 0:n], in0=pp[:, 0:n],
                                            in1=t1[:, 0:n], op=mybir.AluOpType.add)
                    nc.sync.dma_start(out[128 * ot:128 * (ot + 1), t0:t0 + n],
                                      o_t[:, 0:n])

            emit_attention(0)
            emit_qk(1)       # traced after pair0 attention -> PE filler
            emit_attention(1, tail=emit_proj)

    nc.compile()
    return nc


def _shard(inputs):
    x = np.ascontiguousarray(np.asarray(inputs["x"], dtype=np.float32))
    gn_w = np.asarray(inputs["gn_w"], dtype=np.float32)
    gn_b = np.asarray(inputs["gn_b"], dtype=np.float32)
    qkv_w = np.asarray(inputs["qkv_w"], dtype=np.float32)
    qkv_b = np.asarray(inputs["qkv_b"], dtype=np.float32)
    proj_w = np.asarray(inputs["proj_w"], dtype=np.float32)
    proj_b = np.asarray(inputs["proj_b"], dtype=np.float32)

    ind = np.zeros((128, 8), np.float32)
    for c in range(128):
        ind[c, c // GS] = 1.0
    indT = np.ascontiguousarray(ind.T)
    gnw4 = np.ascontiguousarray(gn_w.reshape(CT, 128))
    gnb4 = np.ascontiguousarray(gn_b.reshape(CT, 128))

    in_maps = []
    for core in range(N_CORES):
        b, hg = core // 2, core % 2
        xb = np.ascontiguousarray(x[b].reshape(C, T))

        qkwT = np.empty((C, 512), np.float32)
        qkbm = np.empty((4, 128), np.float32)
        for p_ in range(2):
            for part in range(2):
                for j in range(2):
                    gh = 4 * hg + 2 * p_ + j
                    rows = qkv_w[192 * gh + 64 * part:192 * gh + 64 * part + 64, :]
                    col0 = 256 * p_ + 128 * part + 64 * j
                    qkwT[:, col0:col0 + 64] = (rows * S_ATT).T
                    qkbm[2 * p_ + part, 64 * j:64 * j + 64] = \
                        qkv_b[192 * gh + 64 * part:192 * gh + 64 * part + 64] * S_ATT

        vwT = np.empty((C, 256), np.float32)
        bv = np.empty((256,), np.float32)
        for j in range(HPC):
            gh = 4 * hg + j
            vwT[:, 64 * j:64 * j + 64] = qkv_w[192 * gh + 128:192 * gh + 192, :].T
            bv[64 * j:64 * j + 64] = qkv_b[192 * gh + 128:192 * gh + 192]

        pw = proj_w[:, 256 * hg:256 * hg + 256]
        projwT = np.ascontiguousarray(pw.T)
        pb = pw @ bv
        if hg == 0:
            pb = pb + proj_b
        rs = np.full((CT, 128), 1.0 if hg == 0 else 0.0, np.float32)

        in_maps.append({
            "xb": xb,
            "qkwT": np.ascontiguousarray(qkwT),
            "vwT": np.ascontiguousarray(vwT),
            "projwT": projwT,
            "qkb": np.ascontiguousarray(qkbm),
            "gnw": gnw4, "gnb": gnb4,
            "ind": ind, "indT": indT,
            "resscale": rs,
            "pbeff": np.ascontiguousarray(pb.reshape(CT, 128).astype(np.float32)),
        })
    return in_maps


def _ensure_ntff_hook():
    """Install the antenv.axon_hooks shim so BASS_TRACE=1 can capture NTFF
    profiles through libaxon_pjrt.so (the image ships the .so but not the
    python-side hook module)."""
    try:
        from antenv.axon_hooks import get_axon_ntff_profile_hook  # noqa: F401
        return
    except ImportError:
        pass
    import types

    try:
        import antenv
        from trn_agent_boot.trn_boot import _ntff_profile_via_ctypes
    except ImportError:
        return
    mod = types.ModuleType("antenv.axon_hooks")
    holder = {"hook": None}
    mod.set_axon_ntff_profile_hook = lambda h: holder.__setitem__("hook", h)
    mod.get_axon_ntff_profile_hook = lambda: holder["hook"]
    sys.modules["antenv.axon_hooks"] = mod
    antenv.axon_hooks = mod
    so = "/opt/axon/libaxon_pjrt.so"
    if os.path.exists(so):
        hook = _ntff_profile_via_ctypes(so)
        if hook is not None:
            mod.set_axon_ntff_profile_hook(hook)


LAST_RESULTS = None


def kernel(**inputs):
    global LAST_RESULTS
    if os.environ.get("BASS_TRACE"):
        _ensure_ntff_hook()
    if "nc" not in _CACHE:
        _CACHE["nc"] = _build()
    nc = _CACHE["nc"]
    in_maps = _shard(inputs)
    res = run_bass_kernel_spmd(nc, in_maps, core_ids=list(range(N_CORES)))
    LAST_RESULTS = res
    out = np.empty((B, C, T), np.float32)
    for b in range(B):
        out[b] = res.results[2 * b]["out"] + res.results[2 * b + 1]["out"]
    return out.reshape(B, C, 48, 48)
